# revision 29
# baseline (speedup 1.0000x reference)
"""MetaPathConnector kernel for Trainium2 (8 NeuronCores, Bass/Tile). v4.

Row-shards N=16384 nodes across 8 cores (2048 rows each); each core gets a
rotated featT so its own rows sit at columns [0, 2048).

Algorithm per core:
  prep:  pnat = feat @ W.T per 128-chunk (fp32 LOW_HIGH matmuls, exact),
         row norms via ACT square-accum, inv = 1/sqrt; nh = fp16 normalized
         rows -> DRAM gather table [16384, 256] (row = nh[128] | norm | pad);
         u = round(nrm * 64) as bf16 ints; qT = transposed u (PE transposes).
  main:  per 128-row tile, per 2048-col block: PSUM = u.u (bf16 MMs, integer
         exact) + iota_local*2^-11 (K=2 matmul accumulate).  MAX8 per block
         gives top-8 encoded (value,index) pairs -- no FIND pass over sims.
         Cross-block top-16 via MAX8/match_replace; global column = decoded
         local index + 2048*(slot//8) from FIND_INDEX8 positions in the tiny
         64-wide candidate array.
  refine: dma_gather the 16 candidate rows (fp16, 512B each, 4 DGE queues),
         exact dots s_k = inv_r * (pnat_r . nh_k), top-10 threshold via
         MAX8/match_replace, masked softmax, weighted sum (ACT scales + DVE
         strided reduce), residual out = feat + 0.1*(prop + emb).
"""

from contextlib import ExitStack

import numpy as np

import concourse.bass as bass
import concourse.mybir as mybir
import concourse.tile as tile
from concourse import bacc
from concourse.bass_utils import run_bass_kernel_spmd
from concourse.masks import make_identity

FP32 = mybir.dt.float32
FP16 = mybir.dt.float16
BF16 = mybir.dt.bfloat16
U16 = mybir.dt.uint16
I16 = mybir.dt.int16
AF = mybir.ActivationFunctionType
ALU = mybir.AluOpType

N_NODES = 16384
D = 128
N_CORES = 8
K = 10
C = 24              # gathered candidates per row
STRENGTH = 0.1
BLK = 2048
MMW = 512
GINV = 64.0         # 1/g, g = 2^-6
EPS_I = 2.0 ** -11  # local index encode step
M23 = float(2 ** 23) * 1.5   # round-to-int magic; 1.5x keeps ulp=1 for x<0
SENT = -4096.0
ROWB = 256          # fp16 elems per dram gather row (512B)
NQ = 4              # swdge queues


def build_nc(n_nodes=N_NODES, rows=N_NODES // N_CORES, n_cores=N_CORES,
             stage=9):
    nc = bacc.Bacc("TRN2", target_bir_lowering=False, num_devices=n_cores,
                   num_swdge_queues=NQ)
    featT = nc.dram_tensor("featT", [D, n_nodes], FP32, kind="ExternalInput")
    feat_rows = nc.dram_tensor("feat_rows", [rows, D], FP32,
                               kind="ExternalInput")
    WT = nc.dram_tensor("WT", [D, D], FP32, kind="ExternalInput")
    emb = nc.dram_tensor("emb", [1, D], FP32, kind="ExternalInput")
    iota_hl = nc.dram_tensor("iota_hl", [D, BLK], BF16,
                             kind="ExternalInput")
    cf = nc.dram_tensor("cf", [D, D], BF16, kind="ExternalInput")
    iotaC = nc.dram_tensor("iotaC", [1, C], FP32, kind="ExternalInput")
    out_rows = nc.dram_tensor("out_rows", [rows, D], FP32,
                              kind="ExternalOutput")
    gtab = nc.dram_tensor("gtab", [n_nodes, ROWB], FP16)

    with tile.TileContext(nc) as tc, ExitStack() as ctx:
        _build(ctx, tc, featT.ap(), feat_rows.ap(), WT.ap(), emb.ap(),
               iota_hl.ap(), cf.ap(), iotaC.ap(), out_rows.ap(), gtab.ap(),
               n_nodes, rows, stage)
    nc.compile()
    return nc


def _build(ctx, tc, featT, feat_rows, WT, emb, iota_hl, cf, iotaC, out_rows,
           gtab, n_nodes, rows, stage):
    nc = tc.nc
    n_blocks = n_nodes // BLK            # 8
    n_tiles = rows // 128                # 16
    nchunks = n_nodes // 128             # 128
    CW = n_blocks * 8                    # 64 candidates pre-select

    consts = ctx.enter_context(tc.tile_pool(name="consts", bufs=1))
    bigbuf = ctx.enter_context(tc.tile_pool(name="bigbuf", bufs=1))
    stream = ctx.enter_context(tc.tile_pool(name="stream", bufs=2))
    small = ctx.enter_context(tc.tile_pool(name="small", bufs=2))
    gpool = ctx.enter_context(tc.tile_pool(name="gpool", bufs=6))
    psum_blk = ctx.enter_context(
        tc.tile_pool(name="psum_blk", bufs=2, space="PSUM"))

    # ---------------- constants ----------------
    ident = consts.tile([128, 128], FP32)
    make_identity(nc, ident[:])
    identb = consts.tile([128, 128], BF16)
    nc.vector.tensor_copy(identb[:], ident[:])
    negI = consts.tile([128, 128], FP32)
    nc.gpsimd.memset(negI[:], 0.0)
    nc.gpsimd.affine_select(
        out=negI[:], in_=negI[:], compare_op=ALU.not_equal, fill=SENT,
        base=0, pattern=[[-1, 128]], channel_multiplier=1)

    emb_bc = consts.tile([128, D], FP32)
    nc.sync.dma_start(out=emb_bc[:], in_=emb.to_broadcast([128, D]))
    emb01 = consts.tile([128, D], FP32)
    nc.scalar.mul(emb01[:], emb_bc[:], STRENGTH)

    WT_sb = consts.tile([D, D], FP32)
    nc.sync.dma_start(out=WT_sb[:], in_=WT)
    hl_sb = consts.tile([D, BLK], BF16)
    nc.sync.dma_start(out=hl_sb[:], in_=iota_hl)
    cf_sb = consts.tile([D, D], BF16)
    nc.sync.dma_start(out=cf_sb[:], in_=cf)
    iC = consts.tile([128, C], FP32)
    nc.sync.dma_start(out=iC[:], in_=iotaC.to_broadcast([128, C]))

    # ---------------- prep ----------------
    pnat = bigbuf.tile([128, n_nodes], FP32)      # proj rows, chunk-major
    qT = bigbuf.tile([128, n_nodes], BF16)        # quantized nrm, transposed
    ssq = consts.tile([128, nchunks], FP32)
    inv = consts.tile([128, nchunks], FP32)

    for s in range(n_blocks):                     # 8 strips of 2048 cols
        fT = stream.tile([128, BLK], FP32, tag="ft")
        nc.sync.dma_start(out=fT[:], in_=featT[:, s * BLK:(s + 1) * BLK])
        for gq in range(4):                       # 4 chunk-groups of 512
            pB = psum_blk.tile([128, BLK], FP32, tag="blk")
            pG = pB[:, 0:MMW]
            for j in range(4):
                c = 16 * s + 4 * gq + j
                nc.tensor.matmul(pG[:, j * 128:(j + 1) * 128],
                                 lhsT=fT[:, (4 * gq + j) * 128:
                                         (4 * gq + j + 1) * 128],
                                 rhs=WT_sb[:], start=True, stop=True)
            nc.scalar.copy(pnat[:, (16 * s + 4 * gq) * 128:
                           (16 * s + 4 * gq + 4) * 128], pG)
            sq = stream.tile([128, MMW], FP32, tag="sq")
            nc.scalar.activation(sq[:], pG, AF.Square)
            nc.vector.tensor_reduce(
                ssq[:, 16 * s + 4 * gq:16 * s + 4 * gq + 4],
                sq[:].rearrange("p (c d) -> p c d", d=128),
                axis=mybir.AxisListType.X, op=ALU.add)

    nrmv = consts.tile([128, nchunks], FP32)
    nc.scalar.sqrt(nrmv[:], ssq[:])
    nc.vector.reciprocal(inv[:], nrmv[:])
    norm16 = consts.tile([128, nchunks], FP16)
    nc.vector.tensor_copy(norm16[:], nrmv[:])
    # gather-table norm column: gtab[node, 128]
    nc.sync.dma_start(
        out=gtab.rearrange("(c p) e -> p c e", p=128)[:, :, 128:129],
        in_=norm16[:].rearrange("p (c o) -> p c o", o=1))

    # nh (fp16 normalized rows) per chunk -> DRAM; u -> transpose -> qT
    for s in range(n_blocks):
        nh = stream.tile([128, BLK], FP16, tag="nh")
        for j in range(16):
            c = 16 * s + j
            nc.scalar.activation(nh[:, j * 128:(j + 1) * 128],
                                 pnat[:, c * 128:(c + 1) * 128],
                                 AF.Copy, scale=inv[:, c:c + 1])
        nc.sync.dma_start(
            out=gtab.rearrange("(c p) e -> p c e", p=128)[
                :, 16 * s:16 * (s + 1), 0:128],
            in_=nh[:].rearrange("p (c d) -> p c d", d=128))
        # u = round(nh * 64) ints (+M/-M round, exact on DVE)
        y = stream.tile([128, BLK], FP32, tag="yy")
        nc.vector.tensor_scalar(out=y[:], in0=nh[:], scalar1=GINV,
                                scalar2=M23, op0=ALU.mult, op1=ALU.add)
        ub = y
        nc.vector.tensor_scalar(out=ub[:], in0=y[:], scalar1=M23,
                                scalar2=None, op0=ALU.subtract)
        for gq in range(4):
            pB2 = psum_blk.tile([128, BLK], FP32, tag="blk")
            pT = pB2[:, 0:MMW]
            for j in range(4):
                nc.tensor.transpose(pT[:, j * 128:(j + 1) * 128],
                                    ub[:, (4 * gq + j) * 128:
                                       (4 * gq + j + 1) * 128], ident[:])
            nc.scalar.copy(qT[:, s * BLK + gq * MMW:s * BLK + (gq + 1) * MMW],
                           pT)

    if stage <= 1:   # debug bail: write feat + emb01
        for t in range(n_tiles):
            ft = small.tile([128, D], FP32, tag="ft2")
            nc.sync.dma_start(out=ft[:], in_=feat_rows[t * 128:(t + 1) * 128])
            o = small.tile([128, D], FP32, tag="oo")
            nc.vector.tensor_add(o[:], ft[:], emb01[:])
            nc.sync.dma_start(out=out_rows[t * 128:(t + 1) * 128], in_=o[:])
        return

    # ---------------- main loop ----------------
    for t in range(n_tiles):
        tq = qT[:, t * 128:(t + 1) * 128]
        cand = small.tile([128, CW], FP32, tag="cand")
        for b in range(n_blocks):
            ps = psum_blk.tile([128, BLK], FP32, tag="blk")
            skip_sims = (stage == 22 and t == 0 and b == 0)
            skip_iota = (stage == 23 and t == 0 and b == 0)
            if not skip_iota:
                for m in range(BLK // MMW):
                    nc.tensor.matmul(
                        ps[:, m * MMW:(m + 1) * MMW], lhsT=cf_sb[:],
                        rhs=hl_sb[:, m * MMW:(m + 1) * MMW],
                        start=True, stop=skip_sims)
            if not skip_sims:
                for m in range(BLK // MMW):
                    nc.tensor.matmul(
                        ps[:, m * MMW:(m + 1) * MMW], lhsT=tq,
                        rhs=qT[:, b * BLK + m * MMW:b * BLK + (m + 1) * MMW],
                        start=skip_iota, stop=(m == BLK // MMW - 1))
            if b == (t * 128) // BLK:
                off = (t * 128) % BLK
                nc.vector.tensor_add(ps[:, off:off + 128],
                                     ps[:, off:off + 128], negI[:])
            if stage in (21, 22, 23) and t == 0 and b == 0:
                dbg = small.tile([128, 128], FP32, tag="dbg")
                nc.vector.tensor_copy(dbg[:], ps[:, 128:256])
                nc.sync.dma_start(out=out_rows[4:132], in_=dbg[:])
                dbg2 = small.tile([2, 128], FP32, tag="dbg2")
                nc.vector.tensor_copy(dbg2[:], hl_sb[0:2, 128:256])
                nc.sync.dma_start(out=out_rows[0:2], in_=dbg2[:])
            nc.vector.max(out=cand[:, b * 8:(b + 1) * 8], in_=ps[:])

        # top-16 + positions
        t8a = small.tile([128, 8], FP32, tag="t8a")
        nc.vector.max(out=t8a[:], in_=cand[:])
        cand2 = small.tile([128, CW], FP32, tag="cand2")
        nc.vector.match_replace(out=cand2[:], in_to_replace=t8a[:],
                                in_values=cand[:], imm_value=SENT)
        t8b = small.tile([128, 8], FP32, tag="t8b")
        nc.vector.max(out=t8b[:], in_=cand2[:])
        posa = small.tile([128, 8], U16, tag="posa")
        nc.vector.max_index(out=posa[:], in_max=t8a[:], in_values=cand[:])
        posb = small.tile([128, 8], U16, tag="posb")
        nc.vector.max_index(out=posb[:], in_max=t8b[:], in_values=cand2[:])
        cand3 = small.tile([128, CW], FP32, tag="cand3")
        nc.vector.match_replace(out=cand3[:], in_to_replace=t8b[:],
                                in_values=cand2[:], imm_value=SENT)
        t8c = small.tile([128, 8], FP32, tag="t8c")
        nc.vector.max(out=t8c[:], in_=cand3[:])
        posc = small.tile([128, 8], U16, tag="posc")
        nc.vector.max_index(out=posc[:], in_max=t8c[:], in_values=cand3[:])

        W16 = small.tile([128, C], FP32, tag="w16")
        nc.vector.tensor_copy(W16[:, 0:8], t8a[:])
        nc.vector.tensor_copy(W16[:, 8:16], t8b[:])
        nc.vector.tensor_copy(W16[:, 16:24], t8c[:])
        P16 = small.tile([128, C], FP32, tag="p16")
        nc.vector.tensor_copy(P16[:, 0:8], posa[:])
        nc.vector.tensor_copy(P16[:, 8:16], posb[:])
        nc.vector.tensor_copy(P16[:, 16:24], posc[:])

        # decode: gidx = min(frac*2^11, 1023) + 1024 + 2048*floor(pos/8)
        yw = small.tile([128, C], FP32, tag="yw")
        nc.vector.tensor_scalar(out=yw[:], in0=W16[:], scalar1=M23,
                                scalar2=M23, op0=ALU.add, op1=ALU.subtract)
        dfr = small.tile([128, C], FP32, tag="dfr")
        nc.vector.tensor_tensor(out=dfr[:], in0=W16[:], in1=yw[:],
                                op=ALU.subtract)
        iloc = small.tile([128, C], FP32, tag="iloc")
        nc.vector.tensor_scalar(out=iloc[:], in0=dfr[:], scalar1=2048.0,
                                scalar2=1023.0, op0=ALU.mult, op1=ALU.min)
        base = small.tile([128, C], FP32, tag="base")
        nc.vector.tensor_scalar(out=base[:], in0=P16[:], scalar1=0.125,
                                scalar2=M23 - 0.4375, op0=ALU.mult,
                                op1=ALU.add)
        nc.vector.tensor_scalar(out=base[:], in0=base[:], scalar1=M23,
                                scalar2=2048.0, op0=ALU.subtract,
                                op1=ALU.mult)
        gidxf = small.tile([128, C], FP32, tag="gidxf")
        nc.vector.tensor_tensor(out=gidxf[:], in0=iloc[:], in1=base[:],
                                op=ALU.add)
        nc.vector.tensor_scalar(out=gidxf[:], in0=gidxf[:], scalar1=1024.0,
                                scalar2=16383.0, op0=ALU.add, op1=ALU.min)
        nc.vector.tensor_scalar(out=gidxf[:], in0=gidxf[:], scalar1=0.0,
                                scalar2=None, op0=ALU.max)
        gidx16 = small.tile([128, C], I16, tag="gidx16")
        nc.vector.tensor_copy(gidx16[:], gidxf[:])

        if stage <= 2 or stage in (21, 22, 23):
            if not (stage in (21, 22, 23) and t == 0):
                ft = small.tile([128, D], FP32, tag="ft2")
                nc.sync.dma_start(out=ft[:],
                                  in_=feat_rows[t * 128:(t + 1) * 128])
                o = small.tile([128, D], FP32, tag="oo")
                nc.vector.tensor_add(o[:], ft[:], emb01[:])
                nc.vector.tensor_add(o[:, 0:C], o[:, 0:C], gidxf[:])
                nc.sync.dma_start(out=out_rows[t * 128:(t + 1) * 128],
                                  in_=o[:])
            continue

        # pack idxw: value for gathered row j=k*128+p sits at
        # idxw[p%16, k*8 + p//16], replicated over the 8 16-partition groups.
        idxw = small.tile([128, C * 8], I16, tag="idxw")
        for h in range(8):
            nc.sync.dma_start(out=idxw[0:16, h:C * 8:8],
                              in_=gidx16[16 * h:16 * (h + 1), :])
        nc.sync.dma_start(out=idxw[16:32, :], in_=idxw[0:16, :])
        nc.sync.dma_start(out=idxw[32:64, :], in_=idxw[0:32, :])
        nc.sync.dma_start(out=idxw[64:128, :], in_=idxw[0:64, :])

        # gather C*128 rows (3 gathers of 8*128)
        Gs = []
        for half in range(3):
            G = gpool.tile([128, 8, ROWB], FP16, tag="gath")
            nc.gpsimd.dma_gather(
                out_ap=G[:], in_ap=gtab,
                idxs_ap=idxw[:, half * 64:(half + 1) * 64],
                num_idxs=8 * 128, num_idxs_reg=8 * 128,
                elem_size=ROWB, queue_num=(3 * t + half) % NQ)
            Gs.append(G)

        # refine dots: s = inv_r * (pnat_r . nh_k) + iotaC
        ptile = pnat[:, t * 128:(t + 1) * 128]
        dots = small.tile([128, C], FP32, tag="dots")
        for half in range(3):
            scr = small.tile([128, 8, 128], FP32, tag="scr")
            nc.gpsimd.tensor_tensor(
                out=scr[:], in0=Gs[half][:, :, 0:128],
                in1=ptile.rearrange("p (o d) -> p o d", o=1).to_broadcast(
                    [128, 8, 128]),
                op=ALU.mult)
            nc.vector.tensor_reduce(dots[:, half * 8:(half + 1) * 8], scr[:],
                                    axis=mybir.AxisListType.X, op=ALU.add)
        s16 = small.tile([128, C], FP32, tag="s16")
        nc.vector.scalar_tensor_tensor(
            out=s16[:], in0=dots[:], scalar=inv[:, t:t + 1], in1=iC[:],
            op0=ALU.mult, op1=ALU.add)

        if stage <= 3:
            ft = small.tile([128, D], FP32, tag="ft2")
            nc.sync.dma_start(out=ft[:], in_=feat_rows[t * 128:(t + 1) * 128])
            o = small.tile([128, D], FP32, tag="oo")
            nc.vector.tensor_add(o[:], ft[:], emb01[:])
            nc.vector.tensor_add(o[:, 0:C], o[:, 0:C], s16[:])
            nc.vector.tensor_add(o[:, 32:32 + C], o[:, 32:32 + C], dots[:])
            nc.vector.tensor_add(o[:, 64:64 + C], o[:, 64:64 + C], W16[:])
            nc.vector.tensor_add(o[:, 96:96 + C], o[:, 96:96 + C], P16[:])
            nc.sync.dma_start(out=out_rows[t * 128:(t + 1) * 128], in_=o[:])
            continue

        # rank-10 threshold
        r8a = small.tile([128, 8], FP32, tag="r8a")
        nc.vector.max(out=r8a[:], in_=s16[:])
        s2 = small.tile([128, C], FP32, tag="s2")
        nc.vector.match_replace(out=s2[:], in_to_replace=r8a[:],
                                in_values=s16[:], imm_value=SENT)
        r8b = small.tile([128, 8], FP32, tag="r8b")
        nc.vector.max(out=r8b[:], in_=s2[:])

        # weights: w = exp(s) * (s >= t10); SC = w * 0.1/Z * norm_k
        E16 = small.tile([128, C], FP32, tag="e16")
        nc.scalar.activation(E16[:], s16[:], AF.Exp)
        mask = small.tile([128, C], FP32, tag="mask")
        nc.vector.tensor_scalar(out=mask[:], in0=s16[:],
                                scalar1=r8b[:, 1:2], scalar2=None,
                                op0=ALU.is_ge)
        wE = small.tile([128, C], FP32, tag="we")
        nc.vector.tensor_tensor(out=wE[:], in0=E16[:], in1=mask[:],
                                op=ALU.mult)
        Z = small.tile([128, 1], FP32, tag="zz")
        nc.vector.tensor_reduce(Z[:], wE[:], axis=mybir.AxisListType.X,
                                op=ALU.add)
        iz = small.tile([128, 1], FP32, tag="iz")
        nc.vector.reciprocal(iz[:], Z[:])
        iz01 = small.tile([128, 1], FP32, tag="iz01")
        nc.vector.tensor_scalar(out=iz01[:], in0=iz[:], scalar1=STRENGTH,
                                scalar2=None, op0=ALU.mult)
        SC = small.tile([128, C], FP32, tag="sc")
        nc.vector.tensor_scalar(out=SC[:], in0=wE[:], scalar1=iz01[:],
                                scalar2=None, op0=ALU.mult)
        for half in range(3):
            nc.vector.tensor_tensor(
                out=SC[:, half * 8:(half + 1) * 8],
                in0=SC[:, half * 8:(half + 1) * 8],
                in1=Gs[half][:, :, 128], op=ALU.mult)

        # apply: scaled = G * SC (ACT per k), then strided reduce over k
        scaled = small.tile([128, C, 128], BF16, tag="scaled")
        for k in range(C):
            nc.scalar.activation(scaled[:, k, :], Gs[k // 8][:, k % 8, 0:128],
                                 AF.Copy, scale=SC[:, k:k + 1])
        red = small.tile([128, D], FP32, tag="red")
        nc.vector.tensor_reduce(
            red[:], scaled[:].rearrange("p k d -> p d k"),
            axis=mybir.AxisListType.X, op=ALU.add)
        ft = small.tile([128, D], FP32, tag="ft2")
        nc.sync.dma_start(out=ft[:], in_=feat_rows[t * 128:(t + 1) * 128])
        o = small.tile([128, D], FP32, tag="oo")
        nc.vector.tensor_add(o[:], ft[:], emb01[:])
        nc.vector.tensor_add(o[:], o[:], red[:])
        nc.sync.dma_start(out=out_rows[t * 128:(t + 1) * 128], in_=o[:])


_NC_CACHE = {}


def _get_nc(n_nodes, rows, n_cores):
    key = (n_nodes, rows, n_cores)
    if key not in _NC_CACHE:
        _NC_CACHE[key] = build_nc(n_nodes, rows, n_cores)
    return _NC_CACHE[key]


def make_in_maps(feat, W, emb, n_cores=N_CORES):
    n = feat.shape[0]
    rows = n // n_cores
    featT = np.ascontiguousarray(feat.T.astype(np.float32))
    WT = np.ascontiguousarray(W.T.astype(np.float32))
    emb = np.ascontiguousarray(emb.astype(np.float32))
    il = np.arange(BLK) - 1024
    h = np.floor(il / 64.0)
    l = il - 64 * h
    iota_hl = np.zeros((D, BLK), np.float32)
    iota_hl[0] = h
    iota_hl[1] = l
    import ml_dtypes
    iota_hl = iota_hl.astype(ml_dtypes.bfloat16)
    cfm = np.zeros((D, D), np.float32)
    cfm[0, :] = 64 * EPS_I
    cfm[1, :] = EPS_I
    cfm = cfm.astype(ml_dtypes.bfloat16)
    iotaC = (np.arange(C) * 2.0 ** -18).astype(np.float32)[None, :]
    maps = []
    for c in range(n_cores):
        maps.append({
            "featT": np.ascontiguousarray(np.roll(featT, -rows * c, axis=1)),
            "feat_rows": np.ascontiguousarray(feat[rows * c:rows * (c + 1)]),
            "WT": WT,
            "emb": emb,
            "iota_hl": iota_hl,
            "cf": cfm,
            "iotaC": iotaC,
        })
    return maps


def kernel(feat, W, emb):
    feat = np.asarray(feat, dtype=np.float32)
    W = np.asarray(W, dtype=np.float32)
    emb = np.asarray(emb, dtype=np.float32)
    n = feat.shape[0]
    rows = n // N_CORES
    nc = _get_nc(n, rows, N_CORES)
    in_maps = make_in_maps(feat, W, emb, N_CORES)
    res = run_bass_kernel_spmd(nc, in_maps, core_ids=list(range(N_CORES)))
    out = np.concatenate([res.results[c]["out_rows"] for c in range(N_CORES)],
                         axis=0)
    return out.astype(np.float32)


# revision 30
# speedup vs baseline: 1.3541x; 1.3541x over previous
"""MetaPathConnector kernel for Trainium2 (8 NeuronCores, Bass/Tile). v4.

Row-shards N=16384 nodes across 8 cores (2048 rows each); each core gets a
rotated featT so its own rows sit at columns [0, 2048).

Algorithm per core:
  prep:  pnat = feat @ W.T per 128-chunk (fp32 LOW_HIGH matmuls, exact),
         row norms via ACT square-accum, inv = 1/sqrt; nh = fp16 normalized
         rows -> DRAM gather table [16384, 256] (row = nh[128] | norm | pad);
         u = round(nrm * 64) as bf16 ints; qT = transposed u (PE transposes).
  main:  per 128-row tile, per 2048-col block: PSUM = u.u (bf16 MMs, integer
         exact) + iota_local*2^-11 (K=2 matmul accumulate).  MAX8 per block
         gives top-8 encoded (value,index) pairs -- no FIND pass over sims.
         Cross-block top-16 via MAX8/match_replace; global column = decoded
         local index + 2048*(slot//8) from FIND_INDEX8 positions in the tiny
         64-wide candidate array.
  refine: dma_gather the 16 candidate rows (fp16, 512B each, 4 DGE queues),
         exact dots s_k = inv_r * (pnat_r . nh_k), top-10 threshold via
         MAX8/match_replace, masked softmax, weighted sum (ACT scales + DVE
         strided reduce), residual out = feat + 0.1*(prop + emb).
"""

from contextlib import ExitStack

import numpy as np

import concourse.bass as bass
import concourse.mybir as mybir
import concourse.tile as tile
from concourse import bacc
from concourse.bass_utils import run_bass_kernel_spmd
from concourse.masks import make_identity

FP32 = mybir.dt.float32
FP16 = mybir.dt.float16
BF16 = mybir.dt.bfloat16
U16 = mybir.dt.uint16
I16 = mybir.dt.int16
AF = mybir.ActivationFunctionType
ALU = mybir.AluOpType

N_NODES = 16384
D = 128
N_CORES = 8
K = 10
C = 16              # gathered candidates per row
STRENGTH = 0.1
BLK = 2048
MMW = 512
GINV = 64.0         # 1/g, g = 2^-6
EPS_I = 2.0 ** -11  # local index encode step
M23 = float(2 ** 23) * 1.5   # round-to-int magic; 1.5x keeps ulp=1 for x<0
SENT = -4096.0
ROWB = 256          # fp16 elems per dram gather row (512B)
NQ = 4              # swdge queues


def build_nc(n_nodes=N_NODES, rows=N_NODES // N_CORES, n_cores=N_CORES,
             stage=9):
    nc = bacc.Bacc("TRN2", target_bir_lowering=False, num_devices=n_cores,
                   num_swdge_queues=NQ)
    featT = nc.dram_tensor("featT", [D, n_nodes], FP32, kind="ExternalInput")
    feat_rows = nc.dram_tensor("feat_rows", [rows, D], FP32,
                               kind="ExternalInput")
    WT = nc.dram_tensor("WT", [D, D], FP32, kind="ExternalInput")
    emb = nc.dram_tensor("emb", [1, D], FP32, kind="ExternalInput")
    iota_hl = nc.dram_tensor("iota_hl", [D, BLK], BF16,
                             kind="ExternalInput")
    cf = nc.dram_tensor("cf", [D, D], BF16, kind="ExternalInput")
    iotaC = nc.dram_tensor("iotaC", [1, C], FP32, kind="ExternalInput")
    out_rows = nc.dram_tensor("out_rows", [rows, D], FP32,
                              kind="ExternalOutput")
    gtab = nc.dram_tensor("gtab", [n_nodes, ROWB], FP16)

    with tile.TileContext(nc) as tc, ExitStack() as ctx:
        _build(ctx, tc, featT.ap(), feat_rows.ap(), WT.ap(), emb.ap(),
               iota_hl.ap(), cf.ap(), iotaC.ap(), out_rows.ap(), gtab.ap(),
               n_nodes, rows, stage)
    nc.compile()
    return nc


def _build(ctx, tc, featT, feat_rows, WT, emb, iota_hl, cf, iotaC, out_rows,
           gtab, n_nodes, rows, stage):
    nc = tc.nc
    n_blocks = n_nodes // BLK            # 8
    n_tiles = rows // 128                # 16
    nchunks = n_nodes // 128             # 128
    CW = n_blocks * 8                    # 64 candidates pre-select

    consts = ctx.enter_context(tc.tile_pool(name="consts", bufs=1))
    bigbuf = ctx.enter_context(tc.tile_pool(name="bigbuf", bufs=1))
    stream = ctx.enter_context(tc.tile_pool(name="stream", bufs=2))
    small = ctx.enter_context(tc.tile_pool(name="small", bufs=2))
    gpool = ctx.enter_context(tc.tile_pool(name="gpool", bufs=6))
    psum_blk = ctx.enter_context(
        tc.tile_pool(name="psum_blk", bufs=2, space="PSUM"))

    # ---------------- constants ----------------
    ident = consts.tile([128, 128], FP32)
    make_identity(nc, ident[:])
    identb = consts.tile([128, 128], BF16)
    nc.vector.tensor_copy(identb[:], ident[:])
    negI = consts.tile([128, 128], FP32)
    nc.gpsimd.memset(negI[:], 0.0)
    nc.gpsimd.affine_select(
        out=negI[:], in_=negI[:], compare_op=ALU.not_equal, fill=SENT,
        base=0, pattern=[[-1, 128]], channel_multiplier=1)

    emb_bc = consts.tile([128, D], FP32)
    nc.sync.dma_start(out=emb_bc[:], in_=emb.to_broadcast([128, D]))
    emb01 = consts.tile([128, D], FP32)
    nc.scalar.mul(emb01[:], emb_bc[:], STRENGTH)

    WT_sb = consts.tile([D, D], FP32)
    nc.sync.dma_start(out=WT_sb[:], in_=WT)
    hl_sb = consts.tile([D, BLK], BF16)
    nc.sync.dma_start(out=hl_sb[:], in_=iota_hl)
    cf_sb = consts.tile([D, D], BF16)
    nc.sync.dma_start(out=cf_sb[:], in_=cf)
    iC = consts.tile([128, C], FP32)
    nc.sync.dma_start(out=iC[:], in_=iotaC.to_broadcast([128, C]))

    # ---------------- prep ----------------
    pnat = bigbuf.tile([128, n_nodes], FP32)      # proj rows, chunk-major
    qT = bigbuf.tile([128, n_nodes], BF16)        # quantized nrm, transposed
    ssq = consts.tile([128, nchunks], FP32)
    inv = consts.tile([128, nchunks], FP32)

    for s in range(n_blocks):                     # 8 strips of 2048 cols
        fT = stream.tile([128, BLK], FP32, tag="ft")
        nc.sync.dma_start(out=fT[:], in_=featT[:, s * BLK:(s + 1) * BLK])
        for gq in range(4):                       # 4 chunk-groups of 512
            pB = psum_blk.tile([128, BLK], FP32, tag="blk")
            pG = pB[:, 0:MMW]
            for j in range(4):
                c = 16 * s + 4 * gq + j
                nc.tensor.matmul(pG[:, j * 128:(j + 1) * 128],
                                 lhsT=fT[:, (4 * gq + j) * 128:
                                         (4 * gq + j + 1) * 128],
                                 rhs=WT_sb[:], start=True, stop=True)
            nc.scalar.copy(pnat[:, (16 * s + 4 * gq) * 128:
                           (16 * s + 4 * gq + 4) * 128], pG)
            sq = stream.tile([128, MMW], FP32, tag="sq")
            nc.scalar.activation(sq[:], pG, AF.Square)
            nc.vector.tensor_reduce(
                ssq[:, 16 * s + 4 * gq:16 * s + 4 * gq + 4],
                sq[:].rearrange("p (c d) -> p c d", d=128),
                axis=mybir.AxisListType.X, op=ALU.add)

    nrmv = consts.tile([128, nchunks], FP32)
    nc.scalar.sqrt(nrmv[:], ssq[:])
    nc.vector.reciprocal(inv[:], nrmv[:])
    norm16 = consts.tile([128, nchunks], FP16)
    nc.vector.tensor_copy(norm16[:], nrmv[:])
    # gather-table norm column: gtab[node, 128]
    nc.sync.dma_start(
        out=gtab.rearrange("(c p) e -> p c e", p=128)[:, :, 128:129],
        in_=norm16[:].rearrange("p (c o) -> p c o", o=1))

    # nh (fp16 normalized rows) per chunk -> DRAM; u -> transpose -> qT
    for s in range(n_blocks):
        nh = stream.tile([128, BLK], FP16, tag="nh")
        for j in range(16):
            c = 16 * s + j
            nc.scalar.activation(nh[:, j * 128:(j + 1) * 128],
                                 pnat[:, c * 128:(c + 1) * 128],
                                 AF.Copy, scale=inv[:, c:c + 1])
        nc.sync.dma_start(
            out=gtab.rearrange("(c p) e -> p c e", p=128)[
                :, 16 * s:16 * (s + 1), 0:128],
            in_=nh[:].rearrange("p (c d) -> p c d", d=128))
        # u = round(nh * 64) ints (+M/-M round, exact on DVE)
        y = stream.tile([128, BLK], FP32, tag="yy")
        nc.vector.tensor_scalar(out=y[:], in0=nh[:], scalar1=GINV,
                                scalar2=M23, op0=ALU.mult, op1=ALU.add)
        ub = y
        nc.vector.tensor_scalar(out=ub[:], in0=y[:], scalar1=M23,
                                scalar2=None, op0=ALU.subtract)
        for gq in range(4):
            pB2 = psum_blk.tile([128, BLK], FP32, tag="blk")
            pT = pB2[:, 0:MMW]
            for j in range(4):
                nc.tensor.transpose(pT[:, j * 128:(j + 1) * 128],
                                    ub[:, (4 * gq + j) * 128:
                                       (4 * gq + j + 1) * 128], ident[:])
            nc.scalar.copy(qT[:, s * BLK + gq * MMW:s * BLK + (gq + 1) * MMW],
                           pT)

    if stage <= 1:   # debug bail: write feat + emb01
        for t in range(n_tiles):
            ft = small.tile([128, D], FP32, tag="ft2")
            nc.sync.dma_start(out=ft[:], in_=feat_rows[t * 128:(t + 1) * 128])
            o = small.tile([128, D], FP32, tag="oo")
            nc.vector.tensor_add(o[:], ft[:], emb01[:])
            nc.sync.dma_start(out=out_rows[t * 128:(t + 1) * 128], in_=o[:])
        return

    # ---------------- main loop ----------------
    for t in range(n_tiles):
        tq = qT[:, t * 128:(t + 1) * 128]
        cand = small.tile([128, CW], FP32, tag="cand")
        for b in range(n_blocks):
            ps = psum_blk.tile([128, BLK], FP32, tag="blk")
            skip_sims = (stage == 22 and t == 0 and b == 0)
            skip_iota = (stage == 23 and t == 0 and b == 0)
            if not skip_iota:
                for m in range(BLK // MMW):
                    nc.tensor.matmul(
                        ps[:, m * MMW:(m + 1) * MMW], lhsT=cf_sb[:],
                        rhs=hl_sb[:, m * MMW:(m + 1) * MMW],
                        start=True, stop=skip_sims)
            if not skip_sims:
                for m in range(BLK // MMW):
                    nc.tensor.matmul(
                        ps[:, m * MMW:(m + 1) * MMW], lhsT=tq,
                        rhs=qT[:, b * BLK + m * MMW:b * BLK + (m + 1) * MMW],
                        start=skip_iota, stop=(m == BLK // MMW - 1))
            if b == (t * 128) // BLK:
                off = (t * 128) % BLK
                nc.vector.tensor_add(ps[:, off:off + 128],
                                     ps[:, off:off + 128], negI[:])
            if stage in (21, 22, 23) and t == 0 and b == 0:
                dbg = small.tile([128, 128], FP32, tag="dbg")
                nc.vector.tensor_copy(dbg[:], ps[:, 128:256])
                nc.sync.dma_start(out=out_rows[4:132], in_=dbg[:])
                dbg2 = small.tile([2, 128], FP32, tag="dbg2")
                nc.vector.tensor_copy(dbg2[:], hl_sb[0:2, 128:256])
                nc.sync.dma_start(out=out_rows[0:2], in_=dbg2[:])
            nc.vector.max(out=cand[:, b * 8:(b + 1) * 8], in_=ps[:])

        # top-16 + positions
        t8a = small.tile([128, 8], FP32, tag="t8a")
        nc.vector.max(out=t8a[:], in_=cand[:])
        cand2 = small.tile([128, CW], FP32, tag="cand2")
        nc.vector.match_replace(out=cand2[:], in_to_replace=t8a[:],
                                in_values=cand[:], imm_value=SENT)
        t8b = small.tile([128, 8], FP32, tag="t8b")
        nc.vector.max(out=t8b[:], in_=cand2[:])
        posa = small.tile([128, 8], U16, tag="posa")
        nc.vector.max_index(out=posa[:], in_max=t8a[:], in_values=cand[:])
        posb = small.tile([128, 8], U16, tag="posb")
        nc.vector.max_index(out=posb[:], in_max=t8b[:], in_values=cand2[:])
        W16 = small.tile([128, C], FP32, tag="w16")
        nc.vector.tensor_copy(W16[:, 0:8], t8a[:])
        nc.vector.tensor_copy(W16[:, 8:16], t8b[:])
        P16 = small.tile([128, C], FP32, tag="p16")
        nc.vector.tensor_copy(P16[:, 0:8], posa[:])
        nc.vector.tensor_copy(P16[:, 8:16], posb[:])

        # decode: gidx = min(frac*2^11, 1023) + 1024 + 2048*floor(pos/8)
        yw = small.tile([128, C], FP32, tag="yw")
        nc.vector.tensor_scalar(out=yw[:], in0=W16[:], scalar1=M23,
                                scalar2=M23, op0=ALU.add, op1=ALU.subtract)
        dfr = small.tile([128, C], FP32, tag="dfr")
        nc.vector.tensor_tensor(out=dfr[:], in0=W16[:], in1=yw[:],
                                op=ALU.subtract)
        iloc = small.tile([128, C], FP32, tag="iloc")
        nc.vector.tensor_scalar(out=iloc[:], in0=dfr[:], scalar1=2048.0,
                                scalar2=1023.0, op0=ALU.mult, op1=ALU.min)
        base = small.tile([128, C], FP32, tag="base")
        nc.vector.tensor_scalar(out=base[:], in0=P16[:], scalar1=0.125,
                                scalar2=M23 - 0.4375, op0=ALU.mult,
                                op1=ALU.add)
        nc.vector.tensor_scalar(out=base[:], in0=base[:], scalar1=M23,
                                scalar2=2048.0, op0=ALU.subtract,
                                op1=ALU.mult)
        gidxf = small.tile([128, C], FP32, tag="gidxf")
        nc.vector.tensor_tensor(out=gidxf[:], in0=iloc[:], in1=base[:],
                                op=ALU.add)
        nc.vector.tensor_scalar(out=gidxf[:], in0=gidxf[:], scalar1=1024.0,
                                scalar2=16383.0, op0=ALU.add, op1=ALU.min)
        nc.vector.tensor_scalar(out=gidxf[:], in0=gidxf[:], scalar1=0.0,
                                scalar2=None, op0=ALU.max)
        gidx16 = small.tile([128, C], I16, tag="gidx16")
        nc.vector.tensor_copy(gidx16[:], gidxf[:])

        if stage <= 2 or stage in (21, 22, 23):
            if not (stage in (21, 22, 23) and t == 0):
                ft = small.tile([128, D], FP32, tag="ft2")
                nc.sync.dma_start(out=ft[:],
                                  in_=feat_rows[t * 128:(t + 1) * 128])
                o = small.tile([128, D], FP32, tag="oo")
                nc.vector.tensor_add(o[:], ft[:], emb01[:])
                nc.vector.tensor_add(o[:, 0:C], o[:, 0:C], gidxf[:])
                nc.sync.dma_start(out=out_rows[t * 128:(t + 1) * 128],
                                  in_=o[:])
            continue

        # pack idxw: value for gathered row j=k*128+p sits at
        # idxw[p%16, k*8 + p//16], replicated over the 8 16-partition groups.
        idxw = small.tile([128, C * 8], I16, tag="idxw")
        for h in range(8):
            nc.sync.dma_start(out=idxw[0:16, h:C * 8:8],
                              in_=gidx16[16 * h:16 * (h + 1), :])
        nc.sync.dma_start(out=idxw[16:32, :], in_=idxw[0:16, :])
        nc.sync.dma_start(out=idxw[32:64, :], in_=idxw[0:32, :])
        nc.sync.dma_start(out=idxw[64:128, :], in_=idxw[0:64, :])

        # gather C*128 rows (2 gathers of 8*128)
        Gs = []
        for half in range(2):
            G = gpool.tile([128, 8, ROWB], FP16, tag="gath")
            nc.gpsimd.dma_gather(
                out_ap=G[:], in_ap=gtab,
                idxs_ap=idxw[:, half * 64:(half + 1) * 64],
                num_idxs=8 * 128, num_idxs_reg=8 * 128,
                elem_size=ROWB, queue_num=(2 * t + half) % NQ)
            Gs.append(G)

        # refine dots: s = inv_r * (pnat_r . nh_k) + iotaC
        ptile = pnat[:, t * 128:(t + 1) * 128]
        dots = small.tile([128, C], FP32, tag="dots")
        for half in range(2):
            scr = small.tile([128, 8, 128], FP32, tag="scr")
            nc.vector.tensor_tensor(
                out=scr[:], in0=Gs[half][:, :, 0:128],
                in1=ptile.rearrange("p (o d) -> p o d", o=1).to_broadcast(
                    [128, 8, 128]),
                op=ALU.mult)
            nc.vector.tensor_reduce(dots[:, half * 8:(half + 1) * 8], scr[:],
                                    axis=mybir.AxisListType.X, op=ALU.add)
        s16 = small.tile([128, C], FP32, tag="s16")
        nc.vector.scalar_tensor_tensor(
            out=s16[:], in0=dots[:], scalar=inv[:, t:t + 1], in1=iC[:],
            op0=ALU.mult, op1=ALU.add)

        if stage <= 3:
            ft = small.tile([128, D], FP32, tag="ft2")
            nc.sync.dma_start(out=ft[:], in_=feat_rows[t * 128:(t + 1) * 128])
            o = small.tile([128, D], FP32, tag="oo")
            nc.vector.tensor_add(o[:], ft[:], emb01[:])
            nc.vector.tensor_add(o[:, 0:C], o[:, 0:C], s16[:])
            nc.vector.tensor_add(o[:, 32:32 + C], o[:, 32:32 + C], dots[:])
            nc.vector.tensor_add(o[:, 64:64 + C], o[:, 64:64 + C], W16[:])
            nc.vector.tensor_add(o[:, 96:96 + C], o[:, 96:96 + C], P16[:])
            nc.sync.dma_start(out=out_rows[t * 128:(t + 1) * 128], in_=o[:])
            continue

        # rank-10 threshold
        r8a = small.tile([128, 8], FP32, tag="r8a")
        nc.vector.max(out=r8a[:], in_=s16[:])
        s2 = small.tile([128, C], FP32, tag="s2")
        nc.vector.match_replace(out=s2[:], in_to_replace=r8a[:],
                                in_values=s16[:], imm_value=SENT)
        r8b = small.tile([128, 8], FP32, tag="r8b")
        nc.vector.max(out=r8b[:], in_=s2[:])

        # weights: w = exp(s) * (s >= t10); SC = w * 0.1/Z * norm_k
        E16 = small.tile([128, C], FP32, tag="e16")
        nc.scalar.activation(E16[:], s16[:], AF.Exp)
        mask = small.tile([128, C], FP32, tag="mask")
        nc.vector.tensor_scalar(out=mask[:], in0=s16[:],
                                scalar1=r8b[:, 1:2], scalar2=None,
                                op0=ALU.is_ge)
        wE = small.tile([128, C], FP32, tag="we")
        nc.vector.tensor_tensor(out=wE[:], in0=E16[:], in1=mask[:],
                                op=ALU.mult)
        Z = small.tile([128, 1], FP32, tag="zz")
        nc.vector.tensor_reduce(Z[:], wE[:], axis=mybir.AxisListType.X,
                                op=ALU.add)
        iz = small.tile([128, 1], FP32, tag="iz")
        nc.vector.reciprocal(iz[:], Z[:])
        iz01 = small.tile([128, 1], FP32, tag="iz01")
        nc.vector.tensor_scalar(out=iz01[:], in0=iz[:], scalar1=STRENGTH,
                                scalar2=None, op0=ALU.mult)
        SC = small.tile([128, C], FP32, tag="sc")
        nc.vector.tensor_scalar(out=SC[:], in0=wE[:], scalar1=iz01[:],
                                scalar2=None, op0=ALU.mult)
        for half in range(2):
            nc.vector.tensor_tensor(
                out=SC[:, half * 8:(half + 1) * 8],
                in0=SC[:, half * 8:(half + 1) * 8],
                in1=Gs[half][:, :, 128], op=ALU.mult)

        # apply: scaled = G * SC (ACT per k), then strided reduce over k
        scaled = small.tile([128, C, 128], BF16, tag="scaled")
        for k in range(C):
            nc.scalar.activation(scaled[:, k, :], Gs[k // 8][:, k % 8, 0:128],
                                 AF.Copy, scale=SC[:, k:k + 1])
        red = small.tile([128, D], FP32, tag="red")
        nc.vector.tensor_reduce(
            red[:], scaled[:].rearrange("p k d -> p d k"),
            axis=mybir.AxisListType.X, op=ALU.add)
        ft = small.tile([128, D], FP32, tag="ft2")
        nc.sync.dma_start(out=ft[:], in_=feat_rows[t * 128:(t + 1) * 128])
        o = small.tile([128, D], FP32, tag="oo")
        nc.vector.tensor_add(o[:], ft[:], emb01[:])
        nc.vector.tensor_add(o[:], o[:], red[:])
        nc.sync.dma_start(out=out_rows[t * 128:(t + 1) * 128], in_=o[:])


_NC_CACHE = {}


def _get_nc(n_nodes, rows, n_cores):
    key = (n_nodes, rows, n_cores)
    if key not in _NC_CACHE:
        _NC_CACHE[key] = build_nc(n_nodes, rows, n_cores)
    return _NC_CACHE[key]


def make_in_maps(feat, W, emb, n_cores=N_CORES):
    n = feat.shape[0]
    rows = n // n_cores
    featT = np.ascontiguousarray(feat.T.astype(np.float32))
    WT = np.ascontiguousarray(W.T.astype(np.float32))
    emb = np.ascontiguousarray(emb.astype(np.float32))
    il = np.arange(BLK) - 1024
    h = np.floor(il / 64.0)
    l = il - 64 * h
    iota_hl = np.zeros((D, BLK), np.float32)
    iota_hl[0] = h
    iota_hl[1] = l
    import ml_dtypes
    iota_hl = iota_hl.astype(ml_dtypes.bfloat16)
    cfm = np.zeros((D, D), np.float32)
    cfm[0, :] = 64 * EPS_I
    cfm[1, :] = EPS_I
    cfm = cfm.astype(ml_dtypes.bfloat16)
    iotaC = (np.arange(C) * 2.0 ** -18).astype(np.float32)[None, :]
    maps = []
    for c in range(n_cores):
        maps.append({
            "featT": np.ascontiguousarray(np.roll(featT, -rows * c, axis=1)),
            "feat_rows": np.ascontiguousarray(feat[rows * c:rows * (c + 1)]),
            "WT": WT,
            "emb": emb,
            "iota_hl": iota_hl,
            "cf": cfm,
            "iotaC": iotaC,
        })
    return maps


def kernel(feat, W, emb):
    feat = np.asarray(feat, dtype=np.float32)
    W = np.asarray(W, dtype=np.float32)
    emb = np.asarray(emb, dtype=np.float32)
    n = feat.shape[0]
    rows = n // N_CORES
    nc = _get_nc(n, rows, N_CORES)
    in_maps = make_in_maps(feat, W, emb, N_CORES)
    res = run_bass_kernel_spmd(nc, in_maps, core_ids=list(range(N_CORES)))
    out = np.concatenate([res.results[c]["out_rows"] for c in range(N_CORES)],
                         axis=0)
    return out.astype(np.float32)


# revision 31
# speedup vs baseline: 1.3991x; 1.0332x over previous
"""MetaPathConnector kernel for Trainium2 (8 NeuronCores, Bass/Tile). v4.

Row-shards N=16384 nodes across 8 cores (2048 rows each); each core gets a
rotated featT so its own rows sit at columns [0, 2048).

Algorithm per core:
  prep:  pnat = feat @ W.T per 128-chunk (fp32 LOW_HIGH matmuls, exact),
         row norms via ACT square-accum, inv = 1/sqrt; nh = fp16 normalized
         rows -> DRAM gather table [16384, 256] (row = nh[128] | norm | pad);
         u = round(nrm * 64) as bf16 ints; qT = transposed u (PE transposes).
  main:  per 128-row tile, per 2048-col block: PSUM = u.u (bf16 MMs, integer
         exact) + iota_local*2^-11 (K=2 matmul accumulate).  MAX8 per block
         gives top-8 encoded (value,index) pairs -- no FIND pass over sims.
         Cross-block top-16 via MAX8/match_replace; global column = decoded
         local index + 2048*(slot//8) from FIND_INDEX8 positions in the tiny
         64-wide candidate array.
  refine: dma_gather the 16 candidate rows (fp16, 512B each, 4 DGE queues),
         exact dots s_k = inv_r * (pnat_r . nh_k), top-10 threshold via
         MAX8/match_replace, masked softmax, weighted sum (ACT scales + DVE
         strided reduce), residual out = feat + 0.1*(prop + emb).
"""

from contextlib import ExitStack

import numpy as np

import concourse.bass as bass
import concourse.mybir as mybir
import concourse.tile as tile
from concourse import bacc
from concourse.bass_utils import run_bass_kernel_spmd
from concourse.masks import make_identity

FP32 = mybir.dt.float32
FP16 = mybir.dt.float16
BF16 = mybir.dt.bfloat16
U16 = mybir.dt.uint16
I16 = mybir.dt.int16
AF = mybir.ActivationFunctionType
ALU = mybir.AluOpType

N_NODES = 16384
D = 128
N_CORES = 8
K = 10
C = 16              # gathered candidates per row
STRENGTH = 0.1
BLK = 2048
MMW = 512
GINV = 64.0         # 1/g, g = 2^-6
EPS_I = 2.0 ** -11  # local index encode step
M23 = float(2 ** 23) * 1.5   # round-to-int magic; 1.5x keeps ulp=1 for x<0
SENT = -4096.0
ROWB = 256          # fp16 elems per dram gather row (512B)
NQ = 4              # swdge queues


def build_nc(n_nodes=N_NODES, rows=N_NODES // N_CORES, n_cores=N_CORES,
             stage=9):
    nc = bacc.Bacc("TRN2", target_bir_lowering=False, num_devices=n_cores,
                   num_swdge_queues=NQ)
    featT = nc.dram_tensor("featT", [D, n_nodes], FP32, kind="ExternalInput")
    feat_rows = nc.dram_tensor("feat_rows", [rows, D], FP32,
                               kind="ExternalInput")
    WT = nc.dram_tensor("WT", [D, D], FP32, kind="ExternalInput")
    emb = nc.dram_tensor("emb", [1, D], FP32, kind="ExternalInput")
    iota_hl = nc.dram_tensor("iota_hl", [D, BLK], BF16,
                             kind="ExternalInput")
    cf = nc.dram_tensor("cf", [D, D], BF16, kind="ExternalInput")
    iotaC = nc.dram_tensor("iotaC", [1, C], FP32, kind="ExternalInput")
    out_rows = nc.dram_tensor("out_rows", [rows, D], FP32,
                              kind="ExternalOutput")
    gtab = nc.dram_tensor("gtab", [n_nodes, ROWB], FP16)

    with tile.TileContext(nc) as tc, ExitStack() as ctx:
        _build(ctx, tc, featT.ap(), feat_rows.ap(), WT.ap(), emb.ap(),
               iota_hl.ap(), cf.ap(), iotaC.ap(), out_rows.ap(), gtab.ap(),
               n_nodes, rows, stage)
    nc.compile()
    return nc


def _build(ctx, tc, featT, feat_rows, WT, emb, iota_hl, cf, iotaC, out_rows,
           gtab, n_nodes, rows, stage):
    nc = tc.nc
    n_blocks = n_nodes // BLK            # 8
    n_tiles = rows // 128                # 16
    nchunks = n_nodes // 128             # 128
    CW = n_blocks * 8                    # 64 candidates pre-select

    consts = ctx.enter_context(tc.tile_pool(name="consts", bufs=1))
    bigbuf = ctx.enter_context(tc.tile_pool(name="bigbuf", bufs=1))
    stream = ctx.enter_context(tc.tile_pool(name="stream", bufs=2))
    small = ctx.enter_context(tc.tile_pool(name="small", bufs=2))
    gpool = ctx.enter_context(tc.tile_pool(name="gpool", bufs=6))
    psum_blk = ctx.enter_context(
        tc.tile_pool(name="psum_blk", bufs=2, space="PSUM"))

    # ---------------- constants ----------------
    ident = consts.tile([128, 128], FP32)
    make_identity(nc, ident[:])
    identb = consts.tile([128, 128], BF16)
    nc.vector.tensor_copy(identb[:], ident[:])
    negI = consts.tile([128, 128], FP32)
    nc.gpsimd.memset(negI[:], 0.0)
    nc.gpsimd.affine_select(
        out=negI[:], in_=negI[:], compare_op=ALU.not_equal, fill=SENT,
        base=0, pattern=[[-1, 128]], channel_multiplier=1)

    emb_bc = consts.tile([128, D], FP32)
    nc.sync.dma_start(out=emb_bc[:], in_=emb.to_broadcast([128, D]))
    emb01 = consts.tile([128, D], FP32)
    nc.scalar.mul(emb01[:], emb_bc[:], STRENGTH)

    WT_sb = consts.tile([D, D], FP32)
    nc.sync.dma_start(out=WT_sb[:], in_=WT)
    hl_sb = consts.tile([D, BLK], BF16)
    nc.sync.dma_start(out=hl_sb[:], in_=iota_hl)
    cf_sb = consts.tile([D, D], BF16)
    nc.sync.dma_start(out=cf_sb[:], in_=cf)
    iC = consts.tile([128, C], FP32)
    nc.sync.dma_start(out=iC[:], in_=iotaC.to_broadcast([128, C]))

    # ---------------- prep ----------------
    pnat = bigbuf.tile([128, n_nodes], FP32)      # proj rows, chunk-major
    qT = bigbuf.tile([128, n_nodes], BF16)        # quantized nrm, transposed
    ssq = consts.tile([128, nchunks], FP32)
    inv = consts.tile([128, nchunks], FP32)

    for s in range(n_blocks):                     # 8 strips of 2048 cols
        fT = stream.tile([128, BLK], FP32, tag="ft")
        nc.sync.dma_start(out=fT[:], in_=featT[:, s * BLK:(s + 1) * BLK])
        for gq in range(4):                       # 4 chunk-groups of 512
            pB = psum_blk.tile([128, BLK], FP32, tag="blk")
            pG = pB[:, 0:MMW]
            for j in range(4):
                c = 16 * s + 4 * gq + j
                nc.tensor.matmul(pG[:, j * 128:(j + 1) * 128],
                                 lhsT=fT[:, (4 * gq + j) * 128:
                                         (4 * gq + j + 1) * 128],
                                 rhs=WT_sb[:], start=True, stop=True)
            nc.scalar.copy(pnat[:, (16 * s + 4 * gq) * 128:
                           (16 * s + 4 * gq + 4) * 128], pG)
            sq = stream.tile([128, MMW], FP32, tag="sq")
            nc.scalar.activation(sq[:], pG, AF.Square)
            nc.vector.tensor_reduce(
                ssq[:, 16 * s + 4 * gq:16 * s + 4 * gq + 4],
                sq[:].rearrange("p (c d) -> p c d", d=128),
                axis=mybir.AxisListType.X, op=ALU.add)

    nrmv = consts.tile([128, nchunks], FP32)
    nc.scalar.sqrt(nrmv[:], ssq[:])
    nc.vector.reciprocal(inv[:], nrmv[:])
    norm16 = consts.tile([128, nchunks], FP16)
    nc.vector.tensor_copy(norm16[:], nrmv[:])
    # gather-table norm column: gtab[node, 128]
    nc.sync.dma_start(
        out=gtab.rearrange("(c p) e -> p c e", p=128)[:, :, 128:129],
        in_=norm16[:].rearrange("p (c o) -> p c o", o=1))

    # nh (fp16 normalized rows) per chunk -> DRAM; u -> transpose -> qT
    for s in range(n_blocks):
        nh = stream.tile([128, BLK], FP16, tag="nh")
        for j in range(16):
            c = 16 * s + j
            nc.scalar.activation(nh[:, j * 128:(j + 1) * 128],
                                 pnat[:, c * 128:(c + 1) * 128],
                                 AF.Copy, scale=inv[:, c:c + 1])
        nc.sync.dma_start(
            out=gtab.rearrange("(c p) e -> p c e", p=128)[
                :, 16 * s:16 * (s + 1), 0:128],
            in_=nh[:].rearrange("p (c d) -> p c d", d=128))
        # u = round(nh * 64) ints (+M/-M round, exact on DVE)
        y = stream.tile([128, BLK], FP32, tag="yy")
        nc.vector.tensor_scalar(out=y[:], in0=nh[:], scalar1=GINV,
                                scalar2=M23, op0=ALU.mult, op1=ALU.add)
        ub = y
        nc.vector.tensor_scalar(out=ub[:], in0=y[:], scalar1=M23,
                                scalar2=None, op0=ALU.subtract)
        for gq in range(4):
            pB2 = psum_blk.tile([128, BLK], FP32, tag="blk")
            pT = pB2[:, 0:MMW]
            for j in range(4):
                nc.tensor.transpose(pT[:, j * 128:(j + 1) * 128],
                                    ub[:, (4 * gq + j) * 128:
                                       (4 * gq + j + 1) * 128], ident[:])
            nc.scalar.copy(qT[:, s * BLK + gq * MMW:s * BLK + (gq + 1) * MMW],
                           pT)

    if stage <= 1:   # debug bail: write feat + emb01
        for t in range(n_tiles):
            ft = small.tile([128, D], FP32, tag="ft2")
            nc.sync.dma_start(out=ft[:], in_=feat_rows[t * 128:(t + 1) * 128])
            o = small.tile([128, D], FP32, tag="oo")
            nc.vector.tensor_add(o[:], ft[:], emb01[:])
            nc.sync.dma_start(out=out_rows[t * 128:(t + 1) * 128], in_=o[:])
        return

    def _refine_apply(t, Gs):
        # refine dots: s = inv_r * (pnat_r . nh_k) + iotaC
        ptile = pnat[:, t * 128:(t + 1) * 128]
        dots = small.tile([128, C], FP32, tag="dots")
        for half in range(2):
            scr = small.tile([128, 8, 128], FP32, tag="scr")
            nc.vector.tensor_tensor(
                out=scr[:], in0=Gs[half][:, :, 0:128],
                in1=ptile.rearrange("p (o d) -> p o d", o=1).to_broadcast(
                    [128, 8, 128]),
                op=ALU.mult)
            nc.vector.tensor_reduce(dots[:, half * 8:(half + 1) * 8], scr[:],
                                    axis=mybir.AxisListType.X, op=ALU.add)
        s16 = small.tile([128, C], FP32, tag="s16")
        nc.vector.scalar_tensor_tensor(
            out=s16[:], in0=dots[:], scalar=inv[:, t:t + 1], in1=iC[:],
            op0=ALU.mult, op1=ALU.add)

        # rank-10 threshold
        r8a = small.tile([128, 8], FP32, tag="r8a")
        nc.vector.max(out=r8a[:], in_=s16[:])
        s2 = small.tile([128, C], FP32, tag="s2")
        nc.vector.match_replace(out=s2[:], in_to_replace=r8a[:],
                                in_values=s16[:], imm_value=SENT)
        r8b = small.tile([128, 8], FP32, tag="r8b")
        nc.vector.max(out=r8b[:], in_=s2[:])

        # weights: w = exp(s) * (s >= t10); SC = w * 0.1/Z * norm_k
        E16 = small.tile([128, C], FP32, tag="e16")
        nc.scalar.activation(E16[:], s16[:], AF.Exp)
        mask = small.tile([128, C], FP32, tag="mask")
        nc.vector.tensor_scalar(out=mask[:], in0=s16[:],
                                scalar1=r8b[:, 1:2], scalar2=None,
                                op0=ALU.is_ge)
        wE = small.tile([128, C], FP32, tag="we")
        nc.vector.tensor_tensor(out=wE[:], in0=E16[:], in1=mask[:],
                                op=ALU.mult)
        Z = small.tile([128, 1], FP32, tag="zz")
        nc.vector.tensor_reduce(Z[:], wE[:], axis=mybir.AxisListType.X,
                                op=ALU.add)
        iz = small.tile([128, 1], FP32, tag="iz")
        nc.vector.reciprocal(iz[:], Z[:])
        iz01 = small.tile([128, 1], FP32, tag="iz01")
        nc.vector.tensor_scalar(out=iz01[:], in0=iz[:], scalar1=STRENGTH,
                                scalar2=None, op0=ALU.mult)
        SC = small.tile([128, C], FP32, tag="sc")
        nc.vector.tensor_scalar(out=SC[:], in0=wE[:], scalar1=iz01[:],
                                scalar2=None, op0=ALU.mult)
        for half in range(2):
            nc.vector.tensor_tensor(
                out=SC[:, half * 8:(half + 1) * 8],
                in0=SC[:, half * 8:(half + 1) * 8],
                in1=Gs[half][:, :, 128], op=ALU.mult)

        # apply: scaled = G * SC (ACT per k), then strided reduce over k
        scaled = small.tile([128, C, 128], BF16, tag="scaled")
        for k in range(C):
            nc.scalar.activation(scaled[:, k, :], Gs[k // 8][:, k % 8, 0:128],
                                 AF.Copy, scale=SC[:, k:k + 1])
        red = small.tile([128, D], FP32, tag="red")
        nc.vector.tensor_reduce(
            red[:], scaled[:].rearrange("p k d -> p d k"),
            axis=mybir.AxisListType.X, op=ALU.add)
        ft = small.tile([128, D], FP32, tag="ft2")
        nc.sync.dma_start(out=ft[:], in_=feat_rows[t * 128:(t + 1) * 128])
        o = small.tile([128, D], FP32, tag="oo")
        nc.vector.tensor_add(o[:], ft[:], emb01[:])
        nc.vector.tensor_add(o[:], o[:], red[:])
        nc.sync.dma_start(out=out_rows[t * 128:(t + 1) * 128], in_=o[:])

    pending = []
    # ---------------- main loop ----------------
    for t in range(n_tiles):
        tq = qT[:, t * 128:(t + 1) * 128]
        cand = small.tile([128, CW], FP32, tag="cand")
        for b in range(n_blocks):
            ps = psum_blk.tile([128, BLK], FP32, tag="blk")
            skip_sims = (stage == 22 and t == 0 and b == 0)
            skip_iota = (stage == 23 and t == 0 and b == 0)
            if not skip_iota:
                for m in range(BLK // MMW):
                    nc.tensor.matmul(
                        ps[:, m * MMW:(m + 1) * MMW], lhsT=cf_sb[:],
                        rhs=hl_sb[:, m * MMW:(m + 1) * MMW],
                        start=True, stop=skip_sims)
            if not skip_sims:
                for m in range(BLK // MMW):
                    nc.tensor.matmul(
                        ps[:, m * MMW:(m + 1) * MMW], lhsT=tq,
                        rhs=qT[:, b * BLK + m * MMW:b * BLK + (m + 1) * MMW],
                        start=skip_iota, stop=(m == BLK // MMW - 1))
            if b == (t * 128) // BLK:
                off = (t * 128) % BLK
                nc.vector.tensor_add(ps[:, off:off + 128],
                                     ps[:, off:off + 128], negI[:])
            if stage in (21, 22, 23) and t == 0 and b == 0:
                dbg = small.tile([128, 128], FP32, tag="dbg")
                nc.vector.tensor_copy(dbg[:], ps[:, 128:256])
                nc.sync.dma_start(out=out_rows[4:132], in_=dbg[:])
                dbg2 = small.tile([2, 128], FP32, tag="dbg2")
                nc.vector.tensor_copy(dbg2[:], hl_sb[0:2, 128:256])
                nc.sync.dma_start(out=out_rows[0:2], in_=dbg2[:])
            nc.vector.max(out=cand[:, b * 8:(b + 1) * 8], in_=ps[:])

        # top-16 + positions
        t8a = small.tile([128, 8], FP32, tag="t8a")
        nc.vector.max(out=t8a[:], in_=cand[:])
        cand2 = small.tile([128, CW], FP32, tag="cand2")
        nc.vector.match_replace(out=cand2[:], in_to_replace=t8a[:],
                                in_values=cand[:], imm_value=SENT)
        t8b = small.tile([128, 8], FP32, tag="t8b")
        nc.vector.max(out=t8b[:], in_=cand2[:])
        posa = small.tile([128, 8], U16, tag="posa")
        nc.vector.max_index(out=posa[:], in_max=t8a[:], in_values=cand[:])
        posb = small.tile([128, 8], U16, tag="posb")
        nc.vector.max_index(out=posb[:], in_max=t8b[:], in_values=cand2[:])
        W16 = small.tile([128, C], FP32, tag="w16")
        nc.vector.tensor_copy(W16[:, 0:8], t8a[:])
        nc.vector.tensor_copy(W16[:, 8:16], t8b[:])
        P16 = small.tile([128, C], FP32, tag="p16")
        nc.vector.tensor_copy(P16[:, 0:8], posa[:])
        nc.vector.tensor_copy(P16[:, 8:16], posb[:])

        # decode: gidx = min(frac*2^11, 1023) + 1024 + 2048*floor(pos/8)
        yw = small.tile([128, C], FP32, tag="yw")
        nc.vector.tensor_scalar(out=yw[:], in0=W16[:], scalar1=M23,
                                scalar2=M23, op0=ALU.add, op1=ALU.subtract)
        dfr = small.tile([128, C], FP32, tag="dfr")
        nc.vector.tensor_tensor(out=dfr[:], in0=W16[:], in1=yw[:],
                                op=ALU.subtract)
        iloc = small.tile([128, C], FP32, tag="iloc")
        nc.vector.tensor_scalar(out=iloc[:], in0=dfr[:], scalar1=2048.0,
                                scalar2=1023.0, op0=ALU.mult, op1=ALU.min)
        base = small.tile([128, C], FP32, tag="base")
        nc.vector.tensor_scalar(out=base[:], in0=P16[:], scalar1=0.125,
                                scalar2=M23 - 0.4375, op0=ALU.mult,
                                op1=ALU.add)
        nc.vector.tensor_scalar(out=base[:], in0=base[:], scalar1=M23,
                                scalar2=2048.0, op0=ALU.subtract,
                                op1=ALU.mult)
        gidxf = small.tile([128, C], FP32, tag="gidxf")
        nc.vector.tensor_tensor(out=gidxf[:], in0=iloc[:], in1=base[:],
                                op=ALU.add)
        nc.vector.tensor_scalar(out=gidxf[:], in0=gidxf[:], scalar1=1024.0,
                                scalar2=16383.0, op0=ALU.add, op1=ALU.min)
        nc.vector.tensor_scalar(out=gidxf[:], in0=gidxf[:], scalar1=0.0,
                                scalar2=None, op0=ALU.max)
        gidx16 = small.tile([128, C], I16, tag="gidx16")
        nc.vector.tensor_copy(gidx16[:], gidxf[:])

        if stage <= 2 or stage in (21, 22, 23):
            if not (stage in (21, 22, 23) and t == 0):
                ft = small.tile([128, D], FP32, tag="ft2")
                nc.sync.dma_start(out=ft[:],
                                  in_=feat_rows[t * 128:(t + 1) * 128])
                o = small.tile([128, D], FP32, tag="oo")
                nc.vector.tensor_add(o[:], ft[:], emb01[:])
                nc.vector.tensor_add(o[:, 0:C], o[:, 0:C], gidxf[:])
                nc.sync.dma_start(out=out_rows[t * 128:(t + 1) * 128],
                                  in_=o[:])
            continue

        # pack idxw: value for gathered row j=k*128+p sits at
        # idxw[p%16, k*8 + p//16], replicated over the 8 16-partition groups.
        idxw = small.tile([128, C * 8], I16, tag="idxw")
        for h in range(8):
            nc.sync.dma_start(out=idxw[0:16, h:C * 8:8],
                              in_=gidx16[16 * h:16 * (h + 1), :])
        nc.sync.dma_start(out=idxw[16:32, :], in_=idxw[0:16, :])
        nc.sync.dma_start(out=idxw[32:64, :], in_=idxw[0:32, :])
        nc.sync.dma_start(out=idxw[64:128, :], in_=idxw[0:64, :])

        # gather C*128 rows (2 gathers of 8*128)
        Gs = []
        for half in range(2):
            G = gpool.tile([128, 8, ROWB], FP16, tag="gath")
            nc.gpsimd.dma_gather(
                out_ap=G[:], in_ap=gtab,
                idxs_ap=idxw[:, half * 64:(half + 1) * 64],
                num_idxs=8 * 128, num_idxs_reg=8 * 128,
                elem_size=ROWB, queue_num=(2 * t + half) % NQ)
            Gs.append(G)

        pending.append((t, Gs))
        if len(pending) >= 2 or t == n_tiles - 1:
            for (tp, Gp) in list(pending if t == n_tiles - 1 else
                                 pending[:len(pending) - 1]):
                _refine_apply(tp, Gp)
                pending.remove((tp, Gp))

    for (tp, Gp) in pending:
        _refine_apply(tp, Gp)


_NC_CACHE = {}


def _get_nc(n_nodes, rows, n_cores):
    key = (n_nodes, rows, n_cores)
    if key not in _NC_CACHE:
        _NC_CACHE[key] = build_nc(n_nodes, rows, n_cores)
    return _NC_CACHE[key]


def make_in_maps(feat, W, emb, n_cores=N_CORES):
    n = feat.shape[0]
    rows = n // n_cores
    featT = np.ascontiguousarray(feat.T.astype(np.float32))
    WT = np.ascontiguousarray(W.T.astype(np.float32))
    emb = np.ascontiguousarray(emb.astype(np.float32))
    il = np.arange(BLK) - 1024
    h = np.floor(il / 64.0)
    l = il - 64 * h
    iota_hl = np.zeros((D, BLK), np.float32)
    iota_hl[0] = h
    iota_hl[1] = l
    import ml_dtypes
    iota_hl = iota_hl.astype(ml_dtypes.bfloat16)
    cfm = np.zeros((D, D), np.float32)
    cfm[0, :] = 64 * EPS_I
    cfm[1, :] = EPS_I
    cfm = cfm.astype(ml_dtypes.bfloat16)
    iotaC = (np.arange(C) * 2.0 ** -18).astype(np.float32)[None, :]
    maps = []
    for c in range(n_cores):
        maps.append({
            "featT": np.ascontiguousarray(np.roll(featT, -rows * c, axis=1)),
            "feat_rows": np.ascontiguousarray(feat[rows * c:rows * (c + 1)]),
            "WT": WT,
            "emb": emb,
            "iota_hl": iota_hl,
            "cf": cfm,
            "iotaC": iotaC,
        })
    return maps


def kernel(feat, W, emb):
    feat = np.asarray(feat, dtype=np.float32)
    W = np.asarray(W, dtype=np.float32)
    emb = np.asarray(emb, dtype=np.float32)
    n = feat.shape[0]
    rows = n // N_CORES
    nc = _get_nc(n, rows, N_CORES)
    in_maps = make_in_maps(feat, W, emb, N_CORES)
    res = run_bass_kernel_spmd(nc, in_maps, core_ids=list(range(N_CORES)))
    out = np.concatenate([res.results[c]["out_rows"] for c in range(N_CORES)],
                         axis=0)
    return out.astype(np.float32)
